# revision 71
# baseline (speedup 1.0000x reference)
"""Trainium2 Bass kernel for nn_DS_Attention_7636451852327.

Data-parallel over batch: 32 batches -> 8 NeuronCores, 4 batches (2048 tokens)
per core, 16 token-tiles of 128.

Host-side prep: q/v shipped pre-transposed ([512, T] fp16) so the QKV matmul
lhsT tiles are direct DMA loads (no on-device cast / PE transpose / PSUM
copy).  lin_w rows are permuted so the attention output is written in
(h, d, q) order, and the output bias is folded into the final matmul via an
appended ones-row.

Engine split (vertical, by head, with per-stage knobs): DVE runs the
front-end (QK products / partial-sum tree) for all 8 heads; heads [HD, 8)
then cross to the Pool (GPSIMD) engine from emult onward (emult, rowsum,
1/rowsum, E-row-0 chains, PV products + k-sum tree, u-path, normalize,
corrections).  Pool only ever consumes DVE/ACT-produced data -- DVE never
waits on Pool mid-tile -- and tiles crossing the engine boundary are
double-buffered, so the engines pipeline about a quarter tile apart with
both >95% busy in steady state.  ACT does PSUM evictions and the exp()s.

The PV stage uses a single fused E tile (ea*eb over all 24 key-joints in
one op) so each backend is one products op + a 24->12->6->2->1 add tree per
head group, keeping every big op in the DVE 2x_1p perf mode.  Pool is the
binding engine in steady state, so its launch-heavy small ops (the P-side
1/rowsum and u-path/du chain) run in DVE's slack window instead: Pool's
rowsum lands ~30us before DVE reads it, and Pool consumes rP16/duP only at
its corr tail, so the crossings never stall either engine.  (Divide and
InstPool on Pool are rejected by the neuronxcc engine checks, and an ACT
exp(-ln r) reciprocal thrashes 1.3us activation-table loads twice per
tile -- hence the DVE placement.)
"""
import os as _os
import numpy as np
from contextlib import ExitStack

import concourse.bass as bass
import concourse.mybir as mybir
import concourse.tile as tile
from concourse import bacc
from concourse.bass_utils import run_bass_kernel_spmd
from concourse.masks import make_identity

hp = mybir.dt.float16
f32 = mybir.dt.float32
AL = mybir.AluOpType
AX = mybir.AxisListType
AF = mybir.ActivationFunctionType

P = 128
H = 8
QJ = KJ = 24
D = 6
NQK = QJ * D              # 144
NVA = H * NQK             # 1152
NP = QJ * KJ              # 576 (q,k) pairs per head
DH = 1176                 # 147*8
D_MODEL = 512
W_TOT = 3 * NVA + H * 3   # 3480
B = 32
N = 512
N_CORES = 8
TT = (B // N_CORES) * N // P   # 16 token tiles per core

# custom-weighting chain levels: dst col range <- src col range (per head)
CH_LEVELS = (((6, 7), (3, 4)), ((9, 10), (6, 7)),
             ((12, 15), (9, 10)), ((15, 18), (12, 15)))

# q/k chunks ordered so the Pool-side heads' columns (>=720 within each of
# qa/ka) evict first: their exps gate Pool's whole tile.
QKV_CHUNKS = [
    (512, 1024, 0), (1024, 1152, 0), (1664, 2176, 1), (2176, 2304, 1),
    (0, 512, 0), (1152, 1664, 1),
    (2304, 2816, 2), (2816, 3328, 2), (3328, 3480, 2),
]


def _cfg(name, default):
    v = _os.environ.get(name)
    return int(v) if v else default

HD = _cfg("HD", 5)        # heads [HD, 8) cross to Pool from emult onward
HP = H - HD
XPROD = _cfg("XPROD", 0)  # PV products of head HD-1 also on Pool
EMP = _cfg("EMP", 1)      # P-side emult on pool
RSP = _cfg("RSP", 1)      # P-side rowsum on pool
UPP = _cfg("UPP", 1)      # P-side recip/u/u2/du/chains-u2 on pool
CEP = _cfg("CEP", 1)      # P-side E-row0 chains on pool
LASTP = _cfg("LASTP", 2)  # last tile: 0=no pool, 1=reduced pool share, 2=full


def build_program(tt=TT, inner_repeat=1):
    nc = bacc.Bacc("TRN2", target_bir_lowering=False, debug=False)
    T = tt * P
    qT_dram = nc.dram_tensor("qT", [D_MODEL, T], hp, kind="ExternalInput").ap()
    vT_dram = nc.dram_tensor("vT", [D_MODEL, T], hp, kind="ExternalInput").ap()
    wcat_dram = nc.dram_tensor("w_cat", [D_MODEL, W_TOT], hp, kind="ExternalInput").ap()
    lw_dram = nc.dram_tensor("lin_w", [1184, D_MODEL], hp, kind="ExternalInput").ap()
    out_dram = nc.dram_tensor("out", [T, D_MODEL], f32, kind="ExternalOutput").ap()

    dve, pool, act = nc.vector, nc.gpsimd, nc.scalar

    with tile.TileContext(nc) as tc, ExitStack() as ctx:
        const = ctx.enter_context(tc.tile_pool(name="const", bufs=1))
        wpool = ctx.enter_context(tc.tile_pool(name="wpool", bufs=1))
        io = ctx.enter_context(tc.tile_pool(name="io", bufs=2))
        qkv = ctx.enter_context(tc.tile_pool(name="qkv", bufs=1))
        vab = ctx.enter_context(tc.tile_pool(name="vab", bufs=2))
        vpt = ctx.enter_context(tc.tile_pool(name="vpt", bufs=2))
        bigD = ctx.enter_context(tc.tile_pool(name="bigD", bufs=1))
        bigP = ctx.enter_context(tc.tile_pool(name="bigP", bufs=1))
        xb = ctx.enter_context(tc.tile_pool(name="xb", bufs=2))
        small = ctx.enter_context(tc.tile_pool(name="small", bufs=1))
        smx = ctx.enter_context(tc.tile_pool(name="smx", bufs=2))
        vt = ctx.enter_context(tc.tile_pool(name="vt", bufs=1))
        outp = ctx.enter_context(tc.tile_pool(name="outp", bufs=2))
        ps_t = ctx.enter_context(tc.tile_pool(name="ps_t", bufs=2, space="PSUM"))
        ps_mm = ctx.enter_context(tc.tile_pool(name="ps_mm", bufs=3, space="PSUM"))
        ps_out = ctx.enter_context(tc.tile_pool(name="ps_out", bufs=2, space="PSUM"))

        ident = const.tile([P, P], hp, tag="ident")
        make_identity(nc, ident[:])
        wcat = []
        for k in range(4):
            wk = wpool.tile([P, W_TOT], hp, tag=f"wcat{k}")
            nc.sync.dma_start(wk[:], wcat_dram[k * P:(k + 1) * P, :])
            wcat.append(wk)
        lw = []
        for k in range(10):
            rows = min(P, DH - k * P)
            if k == 9:
                rows += 1  # bias row
            lwk = wpool.tile([P, D_MODEL], hp, tag=f"lw{k}")
            nc.sync.dma_start(lwk[:rows, :], lw_dram[k * P:k * P + rows, :])
            lw.append((lwk, rows))

        pending = None
        for it in range(tt):
          for _rep in range(inner_repeat):
            last = it == tt - 1
            # last-tile engine downgrade: keep Pool busy but shrink its share
            # so the epilogue isn't gated on a long Pool tail.
            def pk(flag):
                return pool if flag else dve
            if last and LASTP == 0:
                pe_em = pe_rs = pe_up = pe_ce = pe_bk = pe_xp = dve
                pe_bp = dve
            elif last and LASTP == 1:
                pe_em, pe_rs, pe_up, pe_ce = pk(EMP), pk(RSP), pk(UPP), pk(CEP)
                pe_bk, pe_xp = dve, pk(XPROD)
                pe_bp = dve
            elif last and LASTP == 3:
                # split the last tile's P backend: products Pool, tree DVE
                pe_em, pe_rs, pe_up, pe_ce = pk(EMP), pk(RSP), pk(UPP), pk(CEP)
                pe_bk, pe_xp = dve, pk(XPROD)
                pe_bp = pool
            else:
                pe_em, pe_rs, pe_up, pe_ce = pk(EMP), pk(RSP), pk(UPP), pk(CEP)
                pe_bk, pe_xp = pool, pk(XPROD)
                pe_bp = pool

            # ---- input tiles: direct transposed fp16 loads ----
            xq, xv = [], []
            for src, dst, nm in ((qT_dram, xq, "q"), (vT_dram, xv, "v")):
                for k in range(4):
                    xk = io.tile([P, P], hp, tag=f"x{nm}{k}")
                    nc.sync.dma_start(xk[:], src[k * P:(k + 1) * P, it * P:(it + 1) * P])
                    dst.append(xk)

            # ---- QKV projection: q/k chunks first (ACT copies feed DVE) ----
            qa_all = qkv.tile([P, NVA], hp, tag="qa_all")
            ka_all = qkv.tile([P, NVA], hp, tag="ka_all")
            va_all = vab.tile([P, NVA], hp, tag="va_all")
            vptok = vpt.tile([P, DH + 1], hp, tag="vptok")
            dve.memset(vptok[:, DH:DH + 1], 1.0)  # ones col -> bias row of v'^T

            def qkv_chunk(c0, c1, kind):
                w_n = c1 - c0
                pmm = ps_mm.tile([P, 512], f32, tag="pmm")
                lhs_tiles = xv if kind == 2 else xq
                for k in range(4):
                    nc.tensor.matmul(pmm[:, :w_n], lhs_tiles[k][:], wcat[k][:, c0:c1],
                                     start=(k == 0), stop=(k == 3))
                if kind == 0:
                    act.copy(qa_all[:, c0:c1], pmm[:, :w_n])
                elif kind == 1:
                    act.copy(ka_all[:, c0 - NVA:c1 - NVA], pmm[:, :w_n])
                else:
                    v0, v1 = c0 - 2 * NVA, c1 - 2 * NVA
                    if v1 <= NVA:
                        act.copy(va_all[:, v0:v1], pmm[:, :w_n])
                    else:
                        act.copy(va_all[:, v0:NVA], pmm[:, :NVA - v0])
                        vp = pmm[:, NVA - v0:w_n].rearrange("p (h c) -> p h c", h=H)
                        vp_dst = vptok[:, :DH].rearrange("p (h c) -> p h c", h=H)[:, :, :3]
                        act.copy(vp_dst, vp)

            for (c0, c1, kind) in QKV_CHUNKS[:4]:
                qkv_chunk(c0, c1, kind)

            qa_v = qa_all[:].rearrange("p (h q d) -> p h q d", h=H, q=QJ)
            ka_v = ka_all[:].rearrange("p (h k d) -> p h k d", h=H, k=KJ)
            va_v = va_all[:].rearrange("p (h d k) -> p h d k", h=H, d=D)
            att_all = vptok[:, :DH].rearrange("p (h c) -> p h c", h=H)[:, :, 3:]

            # epilogue(i-1) part 1: PE transposes of previous tile's v'
            if pending is not None:
                pvpt, pit = pending
                pvT = []
                for k in range(10):
                    cols = min(P, DH + 1 - k * P)
                    pst2 = ps_t.tile([P, P], hp, tag="pst2")
                    nc.tensor.transpose(pst2[:cols, :], pvpt[:, k * P:k * P + cols], ident[:])
                    pvT.append((pst2, cols))

            # ---- per-tile tiles ----
            GD1 = max(3, HP)                      # D-side p1 group size (also P p1)
            pbigD = bigD.tile([P, GD1 * NP * D], hp, tag="pbigD")
            pbigP = bigP.tile([P, HP * NP * D], hp, tag="pbigP")
            if XPROD:
                pbigX = xb.tile([P, NP * D], hp, tag="pbigX")
            else:
                pbigX = None
            s2aD = bigD.tile([P, HD * NP * 2], hp, tag="s2aD")
            s2aP = bigD.tile([P, HP * NP * 2], hp, tag="s2aP")
            eaD = bigD.tile([P, HD * NP], hp, tag="eaD")
            ebD = bigD.tile([P, HD * NP], hp, tag="ebD")
            eaP = smx.tile([P, HP * NP], hp, tag="eaP")
            ebP = smx.tile([P, HP * NP], hp, tag="ebP")
            eD = (smx if XPROD else bigD).tile([P, HD * NP], hp, tag="eD")
            eP = bigP.tile([P, HP * NP], hp, tag="eP")
            t12D = bigD.tile([P, GD1 * NQK * 12], hp, tag="t12D")
            t6D = small.tile([P, GD1 * NQK * 6], hp, tag="t6D")
            t2D = small.tile([P, GD1 * NQK * 2], hp, tag="t2D")
            t12P = bigP.tile([P, HP * NQK * 12], hp, tag="t12P")
            t6P = bigP.tile([P, HP * NQK * 6], hp, tag="t6P")
            t2P = bigP.tile([P, HP * NQK * 2], hp, tag="t2P")
            a0P = bigP.tile([P, HP * NQK], hp, tag="a0P")
            ctP = bigP.tile([P, HP * D * D], hp, tag="ctP")
            r12D = small.tile([P, HD * QJ * 12], hp, tag="r12D")
            r6D = small.tile([P, HD * QJ * 6], hp, tag="r6D")
            r2D = small.tile([P, HD * QJ * 2], hp, tag="r2D")
            sD = small.tile([P, HD * QJ], f32, tag="sD")
            r16D = small.tile([P, HD * QJ], hp, tag="r16D")
            r12P = bigP.tile([P, HP * QJ * 12], hp, tag="r12P")
            r6P = bigP.tile([P, HP * QJ * 6], hp, tag="r6P")
            r2P = bigP.tile([P, HP * QJ * 2], hp, tag="r2P")
            sP = smx.tile([P, HP * QJ], f32, tag="sP")
            rP16 = smx.tile([P, HP * QJ], hp, tag="rP16")
            a0D = small.tile([P, HD * NQK], hp, tag="a0D")
            ctD = small.tile([P, HD * D * D], hp, tag="ctD")
            uD = small.tile([P, HD * QJ], hp, tag="uD")
            u2D = small.tile([P, HD * QJ], hp, tag="u2D")
            duD = small.tile([P, HD * QJ], hp, tag="duD")
            uP = bigP.tile([P, HP * QJ], hp, tag="uP")
            u2P = bigP.tile([P, HP * QJ], hp, tag="u2P")
            duP = bigP.tile([P, HP * QJ], hp, tag="duP")
            tmp8D = small.tile([P, H * 3], hp, tag="tmp8D")
            tmp8P = bigP.tile([P, H * 3], hp, tag="tmp8P")

            s2Dv = s2aD[:].rearrange("p (h pr e) -> p h pr e", h=HD, pr=NP)
            s2Pv = s2aP[:].rearrange("p (h pr e) -> p h pr e", h=HP, pr=NP)
            eDv = eD[:].rearrange("p (h q k) -> p h q k", h=HD, q=QJ)
            ePv = eP[:].rearrange("p (h q k) -> p h q k", h=HP, q=QJ)

            def p1_s2a(pb, s2t, g0, g1, o, base, full=None):
                nh = g1 - g0
                p1v = pb[:, o * NP * D:(o + nh) * NP * D].rearrange(
                    "p (h q k d) -> p h q k d", h=nh, q=QJ, k=KJ)
                p1f = pb[:, o * NP * D:(o + nh) * NP * D].rearrange(
                    "p (h pr d) -> p h pr d", h=nh, pr=NP)
                qa_b = qa_v[:, g0:g1].unsqueeze(3).broadcast_to([P, nh, QJ, KJ, D])
                ka_b = ka_v[:, g0:g1].unsqueeze(2).broadcast_to([P, nh, QJ, KJ, D])
                dve.tensor_tensor(p1v, qa_b, ka_b, AL.mult)
                rel = g0 - base
                s2v = s2t[:, rel * NP * 2:(rel + nh) * NP * 2].rearrange(
                    "p (h pr e) -> p h pr e", h=nh, pr=NP)
                dve.tensor_tensor(s2v, p1f[:, :, :, 0:2], p1f[:, :, :, 2:4], AL.add)
                dve.tensor_tensor(s2v, s2v, p1f[:, :, :, 4:6], AL.add)
                if full is not None:
                    # full score sum (strided operands -> 1x mode, but frees
                    # the P side from emult on Pool and halves its exps)
                    fv = full[:, rel * NP:(rel + nh) * NP].rearrange(
                        "p (h pr) -> p h pr", h=nh)
                    dve.tensor_tensor(fv, s2v[:, :, :, 0], s2v[:, :, :, 1], AL.add)

            def emult(eng, ea_t, eb_t, e_t, nh):
                eng.tensor_tensor(e_t[:], ea_t[:], eb_t[:], AL.mult)

            def rowsum(eng, e_t, r12t, r6t, r2t, st, nh):
                fq = nh * QJ
                ev = e_t[:].rearrange("p (f k) -> p f k", f=fq)
                r12v = r12t[:].rearrange("p (f k) -> p f k", f=fq)
                r6v = r6t[:].rearrange("p (f k) -> p f k", f=fq)
                r2v = r2t[:].rearrange("p (f k) -> p f k", f=fq)
                eng.tensor_tensor(r12v, ev[:, :, 0:12], ev[:, :, 12:24], AL.add)
                eng.tensor_tensor(r6v, r12v[:, :, 0:6], r12v[:, :, 6:12], AL.add)
                eng.tensor_tensor(r2v, r6v[:, :, 0:2], r6v[:, :, 2:4], AL.add)
                eng.tensor_tensor(r2v, r2v, r6v[:, :, 4:6], AL.add)
                with nc.allow_low_precision(reason="fp16 attention"):
                    eng.tensor_tensor(st[:], r2v[:, :, 0], r2v[:, :, 1], AL.add)

            def chains(eng, resolve, nh, toff, tmp):
                t3 = tmp[:, toff * 3:(toff + nh) * 3].rearrange("p (h c) -> p h c", h=nh)
                for dsl, ssl in CH_LEVELS:
                    nd = dsl[1] - dsl[0]
                    dst = resolve(dsl[0], dsl[1])
                    srcv = resolve(ssl[0], ssl[1])
                    if ssl[1] - ssl[0] < nd:
                        srcv = srcv.broadcast_to([P, nh, nd])
                    eng.tensor_tensor(t3[:, :, :nd], dst, srcv, AL.add)
                    eng.tensor_scalar_mul(dst, t3[:, :, :nd], 0.5)

            def e_row0_resolver(ev):
                return lambda c0, c1: ev[:, :, 0, c0:c1]

            def flat_resolver(v):
                return lambda c0, c1: v[:, :, c0:c1]

            def backend(eng, pb, e_v, g0, g1, t12t, t6t, t2t, a0t, peng_prod=None):
                """PV products + 24->1 k-sum tree, full k24 with fused E."""
                nh = g1 - g0
                prod_eng = peng_prod if peng_prod is not None else eng
                p2v = pb[:, :nh * NQK * KJ].rearrange(
                    "p (h d q k) -> p h d q k", h=nh, d=D, q=QJ)
                e_b = e_v.unsqueeze(2).broadcast_to([P, nh, D, QJ, KJ])
                va_b = va_v[:, g0:g1].unsqueeze(3).broadcast_to([P, nh, D, QJ, KJ])
                prod_eng.tensor_tensor(p2v, e_b, va_b, AL.mult)
                p2f = pb[:, :nh * NQK * KJ].rearrange(
                    "p (h f k) -> p h f k", h=nh, f=NQK)
                t12v = t12t[:, :nh * NQK * 12].rearrange("p (h f k) -> p h f k", h=nh, f=NQK)
                t6v = t6t[:, :nh * NQK * 6].rearrange("p (h f k) -> p h f k", h=nh, f=NQK)
                t2v = t2t[:, :nh * NQK * 2].rearrange("p (h f k) -> p h f k", h=nh, f=NQK)
                eng.tensor_tensor(t12v, p2f[:, :, :, 0:12], p2f[:, :, :, 12:24], AL.add)
                eng.tensor_tensor(t6v, t12v[:, :, :, 0:6], t12v[:, :, :, 6:12], AL.add)
                eng.tensor_tensor(t2v, t6v[:, :, :, 0:2], t6v[:, :, :, 2:4], AL.add)
                eng.tensor_tensor(t2v, t2v, t6v[:, :, :, 4:6], AL.add)
                eng.tensor_tensor(a0t[:, :nh * NQK].rearrange("p (h f) -> p h f", h=nh),
                                  t2v[:, :, :, 0], t2v[:, :, :, 1], AL.add)

            def corr_att(eng, attv, a0v, r_b, du_t, va0, ct_t, nh, r_op=AL.mult):
                eng.tensor_tensor(attv, a0v, r_b, r_op)
                du_v = du_t[:].rearrange("p (h q) -> p h q", h=nh)
                ctv = ct_t[:].rearrange("p (h d q) -> p h d q", h=nh, d=D)
                for qsl, n_q in (((12, 18, 1), 6), ((6, 10, 3), 2)):
                    du_b = du_v[:, :, qsl[0]:qsl[1]:qsl[2]].unsqueeze(2).broadcast_to(
                        [P, nh, D, n_q])
                    va0_b = va0.unsqueeze(3).broadcast_to([P, nh, D, n_q])
                    eng.tensor_tensor(ctv[:, :, :, :n_q], du_b, va0_b, AL.mult)
                    eng.tensor_tensor(attv[:, :, :, qsl[0]:qsl[1]:qsl[2]],
                                      attv[:, :, :, qsl[0]:qsl[1]:qsl[2]],
                                      ctv[:, :, :, :n_q], AL.add)

            # --- DVE front-end: P side first (advances the Pool start gate) ---
            # (p1 scratch goes in pbigD so pbigP stays Pool-only)
            p1_s2a(pbigD, s2aP, HD, H, 0, HD)
            act.activation(eaP[:], s2Pv[:, :, :, 0], AF.Exp)
            act.activation(ebP[:], s2Pv[:, :, :, 1], AF.Exp)
            # remaining (D-side-only) q/k chunks, behind the P exps on ACT
            for (c0, c1, kind) in QKV_CHUNKS[4:6]:
                qkv_chunk(c0, c1, kind)

            # Pool picks up the P side from emult onward
            emult(pe_em, eaP, ebP, eP, HP)
            rowsum(pe_rs, eP, r12P, r6P, r2P, sP, HP)
            chains(pe_ce, e_row0_resolver(ePv), HP, 0, tmp8P)
            rPv, rP_op = rP16[:].rearrange("p (h q) -> p h q", h=HP), AL.mult

            # --- DVE D-side front-end (boundary head HD-1 first for XPROD) ---
            p1_s2a(pbigD, s2aD, HD - GD1, HD, 0, 0)
            act.activation(eaD[:, (HD - GD1) * NP:], s2Dv[:, HD - GD1:, :, 0], AF.Exp)
            act.activation(ebD[:, (HD - GD1) * NP:], s2Dv[:, HD - GD1:, :, 1], AF.Exp)
            if HD > GD1:
                p1_s2a(pbigD, s2aD, 0, HD - GD1, 0, 0)
                act.activation(eaD[:, :(HD - GD1) * NP], s2Dv[:, :HD - GD1, :, 0], AF.Exp)
                act.activation(ebD[:, :(HD - GD1) * NP], s2Dv[:, :HD - GD1, :, 1], AF.Exp)

            # va chunks (ACT after the exps) -- Pool p2 needs them
            for (c0, c1, kind) in QKV_CHUNKS[6:]:
                qkv_chunk(c0, c1, kind)

            # D-side emult + rowsum (rowsum must see pre-chain E), then chains;
            # split into the p1 groups so DVE starts on g1 while ACT exps g2
            def emult_g(g0, g1):
                dve.tensor_tensor(eD[:, g0 * NP:g1 * NP],
                                  eaD[:, g0 * NP:g1 * NP],
                                  ebD[:, g0 * NP:g1 * NP], AL.mult)

            def rowsum_g(g0, g1):
                fq0, fq1 = g0 * QJ, g1 * QJ
                ev = eD[:, g0 * NP:g1 * NP].rearrange("p (f k) -> p f k", k=KJ)
                r12v = r12D[:, fq0 * 12:fq1 * 12].rearrange("p (f k) -> p f k", k=12)
                r6v = r6D[:, fq0 * 6:fq1 * 6].rearrange("p (f k) -> p f k", k=6)
                r2v = r2D[:, fq0 * 2:fq1 * 2].rearrange("p (f k) -> p f k", k=2)
                dve.tensor_tensor(r12v, ev[:, :, 0:12], ev[:, :, 12:24], AL.add)
                dve.tensor_tensor(r6v, r12v[:, :, 0:6], r12v[:, :, 6:12], AL.add)
                dve.tensor_tensor(r2v, r6v[:, :, 0:2], r6v[:, :, 2:4], AL.add)
                dve.tensor_tensor(r2v, r2v, r6v[:, :, 4:6], AL.add)
                with nc.allow_low_precision(reason="fp16 attention"):
                    dve.tensor_tensor(sD[:, fq0:fq1], r2v[:, :, 0], r2v[:, :, 1],
                                      AL.add)

            emult_g(HD - GD1, HD)
            rowsum_g(HD - GD1, HD)
            if HD > GD1:
                emult_g(0, HD - GD1)
                rowsum_g(0, HD - GD1)
            chains(dve, e_row0_resolver(eDv), HD, 0, tmp8D)

            # epilogue(i-1) part 2: vT copies (ACT), final matmul, out
            if pending is not None:
                vT = []
                for k in range(10):
                    pst2, cols = pvT[k]
                    vk = vt.tile([P, P], hp, tag=f"vT{k}")
                    act.copy(vk[:cols, :], pst2[:cols, :])
                    vT.append((vk, cols))
                pout = ps_out.tile([P, D_MODEL], f32, tag="pout")
                for k in range(10):
                    vk, rows = vT[k]
                    lwk, rows2 = lw[k]
                    nc.tensor.matmul(pout[:], vk[:rows, :], lwk[:rows, :],
                                     start=(k == 0), stop=(k == 9))
                out_sb = outp.tile([P, D_MODEL], f32, tag="out_sb")
                act.copy(out_sb[:], pout[:])
                nc.sync.dma_start(out_dram[pit * P:(pit + 1) * P, :], out_sb[:])

            # Pool backend (P side), full k24
            backend(pe_bk, pbigP, ePv, HD, H, t12P, t6P, t2P, a0P,
                    peng_prod=pe_bp)
            # u-path + P-side 1/r on DVE's slack window: sP/eP are
            # Pool-produced well before DVE reaches this point, and Pool
            # consumes rP16/duP only at its corr tail (Pool is the binding
            # engine here).
            with nc.allow_low_precision(reason="fp16 attention"):
                dve.reciprocal(rP16[:], sP[:])
            dve.tensor_tensor(uP[:].rearrange("p (h q) -> p h q", h=HP),
                              ePv[:, :, :, 0], rPv, AL.mult)
            dve.tensor_copy(u2P[:], uP[:])
            chains(dve, flat_resolver(u2P[:].rearrange("p (h q) -> p h q", h=HP)),
                   HP, HP, tmp8P)
            dve.tensor_tensor(duP[:], u2P[:], uP[:], AL.subtract)
            a0Pv = a0P[:].rearrange("p (h d q) -> p h d q", h=HP, d=D)
            attP = att_all[:, HD:].rearrange("p h (d q) -> p h d q", d=D)
            r_bP = rPv.unsqueeze(2).broadcast_to([P, HP, D, QJ])
            corr_att(pe_bk, attP, a0Pv, r_bP, duP, va_v[:, HD:, :, 0], ctP, HP,
                     r_op=rP_op)

            # --- DVE D-side rest ---
            with nc.allow_low_precision(reason="fp16 attention"):
                dve.reciprocal(r16D[:], sD[:])
            rDv = r16D[:].rearrange("p (h q) -> p h q", h=HD)
            dve.tensor_tensor(uD[:].rearrange("p (h q) -> p h q", h=HD),
                              eDv[:, :, :, 0], rDv, AL.mult)
            dve.tensor_copy(u2D[:], uD[:])
            chains(dve, flat_resolver(u2D[:].rearrange("p (h q) -> p h q", h=HD)),
                   HD, 0, tmp8D)
            dve.tensor_tensor(duD[:], u2D[:], uD[:], AL.subtract)

            # D-side backends: boundary head's products optionally on Pool
            xh = HD - 1
            if XPROD:
                backend(dve, pbigX, eDv[:, xh:HD], xh, HD, t12D, t6D, t2D,
                        a0D[:, xh * NQK:], peng_prod=pe_xp)
            gsz = HD - (1 if XPROD else 0)
            done = 0
            while done < gsz:
                g = min(GD1, gsz - done)
                off = done * NQK
                a0s = a0D[:, off:off + g * NQK]
                backend(dve, pbigD, eDv[:, done:done + g], done, done + g,
                        t12D, t6D, t2D, a0s)
                done += g
            a0Dv = a0D[:].rearrange("p (h d q) -> p h d q", h=HD, d=D)
            attD = att_all[:, :HD].rearrange("p h (d q) -> p h d q", d=D)
            r_bD = rDv.unsqueeze(2).broadcast_to([P, HD, D, QJ])
            corr_att(dve, attD, a0Dv, r_bD, duD, va_v[:, :HD, :, 0], ctD, HD)

            pending = (vptok, it)

        # epilogue flush for the last tile
        pvpt, pit = pending
        vT = []
        for k in range(10):
            cols = min(P, DH + 1 - k * P)
            pst2 = ps_t.tile([P, P], hp, tag="pst2")
            nc.tensor.transpose(pst2[:cols, :], pvpt[:, k * P:k * P + cols], ident[:])
            vk = vt.tile([P, P], hp, tag=f"vT{k}")
            act.copy(vk[:cols, :], pst2[:cols, :])
            vT.append((vk, cols))
        pout = ps_out.tile([P, D_MODEL], f32, tag="pout")
        for k in range(10):
            vk, rows = vT[k]
            lwk, rows2 = lw[k]
            nc.tensor.matmul(pout[:], vk[:rows, :], lwk[:rows, :],
                             start=(k == 0), stop=(k == 9))
        out_sb = outp.tile([P, D_MODEL], f32, tag="out_sb")
        act.copy(out_sb[:], pout[:])
        nc.sync.dma_start(out_dram[pit * P:(pit + 1) * P, :], out_sb[:])

    nc.compile()
    return nc


def prep_weights(qk_w, v_w, lin_w, lin_b):
    scale = np.float32(1.0 / np.sqrt(6.0))
    wq = np.asarray(qk_w[:, :DH], dtype=np.float32).reshape(D_MODEL, H, 147)
    wk = np.asarray(qk_w[:, DH:], dtype=np.float32).reshape(D_MODEL, H, 147)
    wv = np.asarray(v_w, dtype=np.float32).reshape(D_MODEL, H, 147)
    wq_p = (wq[:, :, 3:] * scale).reshape(D_MODEL, H * NQK)
    wk_p = wk[:, :, 3:].reshape(D_MODEL, H * NQK)
    wv_att = wv[:, :, 3:].reshape(D_MODEL, H, KJ, D).transpose(0, 1, 3, 2).reshape(D_MODEL, H * NQK)
    wv_pass = wv[:, :, :3].reshape(D_MODEL, H * 3)
    w_cat = np.ascontiguousarray(
        np.concatenate([wq_p, wk_p, wv_att, wv_pass], axis=1)).astype(np.float16)
    # lin_w rows permuted to the (h, [pass3, d*24+q]) vptok layout + bias row
    lwr = np.asarray(lin_w, dtype=np.float32).reshape(H, 147, D_MODEL)
    att = lwr[:, 3:, :].reshape(H, QJ, D, D_MODEL).transpose(0, 2, 1, 3).reshape(H, NQK, D_MODEL)
    lw_p = np.concatenate([lwr[:, :3, :], att], axis=1).reshape(DH, D_MODEL)
    lw_aug = np.zeros((1184, D_MODEL), dtype=np.float32)
    lw_aug[:DH] = lw_p
    lw_aug[DH] = np.asarray(lin_b, dtype=np.float32)
    return w_cat, np.ascontiguousarray(lw_aug).astype(np.float16)


def make_in_maps(query, value, qk_w, v_w, lin_w, lin_b):
    w_cat, lw_aug = prep_weights(qk_w, v_w, lin_w, lin_b)
    q = np.asarray(query, dtype=np.float32)
    v = np.asarray(value, dtype=np.float32)
    bpc = B // N_CORES
    in_maps = []
    for c in range(N_CORES):
        qc = q[c * bpc:(c + 1) * bpc].reshape(-1, D_MODEL).T
        vc = v[c * bpc:(c + 1) * bpc].reshape(-1, D_MODEL).T
        in_maps.append({
            "qT": np.ascontiguousarray(qc).astype(np.float16),
            "vT": np.ascontiguousarray(vc).astype(np.float16),
            "w_cat": w_cat,
            "lin_w": lw_aug,
        })
    return in_maps


_CACHED_NC = None


def _get_nc():
    global _CACHED_NC
    if _CACHED_NC is None:
        _CACHED_NC = build_program(TT)
    return _CACHED_NC


def kernel(query, key, value, qk_w, v_w, lin_w, lin_b, _want_results=False, **_ignored):
    """Full-input kernel: shards batch over 8 cores, returns full output."""
    in_maps = make_in_maps(query, value, qk_w, v_w, lin_w, lin_b)
    nc = _get_nc()
    bpc = B // N_CORES
    res = run_bass_kernel_spmd(nc, in_maps, core_ids=list(range(N_CORES)))
    out = np.empty((B, N, D_MODEL), dtype=np.float32)
    for c in range(N_CORES):
        out[c * bpc:(c + 1) * bpc] = res.results[c]["out"].reshape(bpc, N, D_MODEL)
    if _want_results:
        return out, res
    return out


# revision 73
# speedup vs baseline: 1.0038x; 1.0038x over previous
"""Trainium2 Bass kernel for nn_DS_Attention_7636451852327.

Data-parallel over batch: 32 batches -> 8 NeuronCores, 4 batches (2048 tokens)
per core, 16 token-tiles of 128.

Host-side prep: q/v shipped pre-transposed ([512, T] fp16) so the QKV matmul
lhsT tiles are direct DMA loads (no on-device cast / PE transpose / PSUM
copy).  lin_w rows are permuted so the attention output is written in
(h, d, q) order, and the output bias is folded into the final matmul via an
appended ones-row.

Engine split (vertical, by head, with per-stage knobs): DVE runs the
front-end (QK products / partial-sum tree) for all 8 heads; heads [HD, 8)
then cross to the Pool (GPSIMD) engine from emult onward (emult, rowsum,
1/rowsum, E-row-0 chains, PV products + k-sum tree, u-path, normalize,
corrections).  Pool only ever consumes DVE/ACT-produced data -- DVE never
waits on Pool mid-tile -- and tiles crossing the engine boundary are
double-buffered, so the engines pipeline about a quarter tile apart with
both >95% busy in steady state.  ACT does PSUM evictions and the exp()s.

The PV stage uses a single fused E tile (ea*eb over all 24 key-joints in
one op) so each backend is one products op + a 24->12->6->2->1 add tree per
head group, keeping every big op in the DVE 2x_1p perf mode.  Pool is the
binding engine in steady state, so its launch-heavy small ops (the P-side
1/rowsum and u-path/du chain) run in DVE's slack window instead: Pool's
rowsum lands ~30us before DVE reads it, and Pool consumes rP16/duP only at
its corr tail, so the crossings never stall either engine.  (Divide and
InstPool on Pool are rejected by the neuronxcc engine checks, and an ACT
exp(-ln r) reciprocal thrashes 1.3us activation-table loads twice per
tile -- hence the DVE placement.)
"""
import os as _os
import numpy as np
from contextlib import ExitStack

import concourse.bass as bass
import concourse.mybir as mybir
import concourse.tile as tile
from concourse import bacc
from concourse.bass_utils import run_bass_kernel_spmd
from concourse.masks import make_identity

hp = mybir.dt.float16
f32 = mybir.dt.float32
AL = mybir.AluOpType
AX = mybir.AxisListType
AF = mybir.ActivationFunctionType

P = 128
H = 8
QJ = KJ = 24
D = 6
NQK = QJ * D              # 144
NVA = H * NQK             # 1152
NP = QJ * KJ              # 576 (q,k) pairs per head
DH = 1176                 # 147*8
D_MODEL = 512
W_TOT = 3 * NVA + H * 3   # 3480
B = 32
N = 512
N_CORES = 8
TT = (B // N_CORES) * N // P   # 16 token tiles per core

# custom-weighting chain levels: dst col range <- src col range (per head)
CH_LEVELS = (((6, 7), (3, 4)), ((9, 10), (6, 7)),
             ((12, 15), (9, 10)), ((15, 18), (12, 15)))

# q/k chunks ordered so the Pool-side heads' columns (>=720 within each of
# qa/ka) evict first: their exps gate Pool's whole tile.
QKV_CHUNKS = [
    (512, 1024, 0), (1024, 1152, 0), (1664, 2176, 1), (2176, 2304, 1),
    (0, 512, 0), (1152, 1664, 1),
    (2304, 2816, 2), (2816, 3328, 2), (3328, 3480, 2),
]


def _cfg(name, default):
    v = _os.environ.get(name)
    return int(v) if v else default

HD = _cfg("HD", 5)        # heads [HD, 8) cross to Pool from emult onward
HP = H - HD
XPROD = _cfg("XPROD", 0)  # PV products of head HD-1 also on Pool
EMP = _cfg("EMP", 1)      # P-side emult on pool
RSP = _cfg("RSP", 1)      # P-side rowsum on pool
UPP = _cfg("UPP", 1)      # P-side recip/u/u2/du/chains-u2 on pool
CEP = _cfg("CEP", 1)      # P-side E-row0 chains on pool
LASTP = _cfg("LASTP", 2)  # last tile: 0=no pool, 1=reduced pool share, 2=full


def build_program(tt=TT, inner_repeat=1):
    nc = bacc.Bacc("TRN2", target_bir_lowering=False, debug=False)
    T = tt * P
    qT_dram = nc.dram_tensor("qT", [D_MODEL, T], hp, kind="ExternalInput").ap()
    vT_dram = nc.dram_tensor("vT", [D_MODEL, T], hp, kind="ExternalInput").ap()
    wcat_dram = nc.dram_tensor("w_cat", [D_MODEL, W_TOT], hp, kind="ExternalInput").ap()
    lw_dram = nc.dram_tensor("lin_w", [1184, D_MODEL], hp, kind="ExternalInput").ap()
    out_dram = nc.dram_tensor("out", [T, D_MODEL], f32, kind="ExternalOutput").ap()

    dve, pool, act = nc.vector, nc.gpsimd, nc.scalar

    with tile.TileContext(nc) as tc, ExitStack() as ctx:
        const = ctx.enter_context(tc.tile_pool(name="const", bufs=1))
        wpool = ctx.enter_context(tc.tile_pool(name="wpool", bufs=1))
        io = ctx.enter_context(tc.tile_pool(name="io", bufs=2))
        qkv = ctx.enter_context(tc.tile_pool(name="qkv", bufs=1))
        vab = ctx.enter_context(tc.tile_pool(name="vab", bufs=2))
        vpt = ctx.enter_context(tc.tile_pool(name="vpt", bufs=2))
        bigD = ctx.enter_context(tc.tile_pool(name="bigD", bufs=1))
        bigP = ctx.enter_context(tc.tile_pool(name="bigP", bufs=1))
        xb = ctx.enter_context(tc.tile_pool(name="xb", bufs=2))
        small = ctx.enter_context(tc.tile_pool(name="small", bufs=1))
        smx = ctx.enter_context(tc.tile_pool(name="smx", bufs=2))
        vt = ctx.enter_context(tc.tile_pool(name="vt", bufs=1))
        outp = ctx.enter_context(tc.tile_pool(name="outp", bufs=2))
        ps_t = ctx.enter_context(tc.tile_pool(name="ps_t", bufs=2, space="PSUM"))
        ps_mm = ctx.enter_context(tc.tile_pool(name="ps_mm", bufs=3, space="PSUM"))
        ps_out = ctx.enter_context(tc.tile_pool(name="ps_out", bufs=2, space="PSUM"))

        ident = const.tile([P, P], hp, tag="ident")
        make_identity(nc, ident[:])
        wcat = []
        for k in range(4):
            wk = wpool.tile([P, W_TOT], hp, tag=f"wcat{k}")
            nc.sync.dma_start(wk[:], wcat_dram[k * P:(k + 1) * P, :])
            wcat.append(wk)
        lw = []
        for k in range(10):
            rows = min(P, DH - k * P)
            if k == 9:
                rows += 1  # bias row
            lwk = wpool.tile([P, D_MODEL], hp, tag=f"lw{k}")
            nc.sync.dma_start(lwk[:rows, :], lw_dram[k * P:k * P + rows, :])
            lw.append((lwk, rows))

        pending = None
        for it in range(tt):
          for _rep in range(inner_repeat):
            last = it == tt - 1
            first = it == 0
            # last-tile engine downgrade: keep Pool busy but shrink its share
            # so the epilogue isn't gated on a long Pool tail.
            def pk(flag):
                return pool if flag else dve
            if last and LASTP == 0:
                pe_em = pe_rs = pe_up = pe_ce = pe_bk = pe_xp = dve
                pe_bp = dve
            elif last and LASTP == 1:
                pe_em, pe_rs, pe_up, pe_ce = pk(EMP), pk(RSP), pk(UPP), pk(CEP)
                pe_bk, pe_xp = dve, pk(XPROD)
                pe_bp = dve
            elif last and LASTP == 3:
                # split the last tile's P backend: products Pool, tree DVE
                pe_em, pe_rs, pe_up, pe_ce = pk(EMP), pk(RSP), pk(UPP), pk(CEP)
                pe_bk, pe_xp = dve, pk(XPROD)
                pe_bp = pool
            else:
                pe_em, pe_rs, pe_up, pe_ce = pk(EMP), pk(RSP), pk(UPP), pk(CEP)
                pe_bk, pe_xp = pool, pk(XPROD)
                pe_bp = pool
            if first:
                # tile 0: Pool starts ~35us late (weight-DMA chain); running
                # the small P-side stages on DVE shrinks Pool's persistent
                # lag, which is what the terminal drain pays for.
                pe_em = pe_rs = pe_ce = dve

            # ---- input tiles: direct transposed fp16 loads ----
            xq, xv = [], []
            for src, dst, nm in ((qT_dram, xq, "q"), (vT_dram, xv, "v")):
                for k in range(4):
                    xk = io.tile([P, P], hp, tag=f"x{nm}{k}")
                    nc.sync.dma_start(xk[:], src[k * P:(k + 1) * P, it * P:(it + 1) * P])
                    dst.append(xk)

            # ---- QKV projection: q/k chunks first (ACT copies feed DVE) ----
            qa_all = qkv.tile([P, NVA], hp, tag="qa_all")
            ka_all = qkv.tile([P, NVA], hp, tag="ka_all")
            va_all = vab.tile([P, NVA], hp, tag="va_all")
            vptok = vpt.tile([P, DH + 1], hp, tag="vptok")
            dve.memset(vptok[:, DH:DH + 1], 1.0)  # ones col -> bias row of v'^T

            def qkv_chunk(c0, c1, kind):
                w_n = c1 - c0
                pmm = ps_mm.tile([P, 512], f32, tag="pmm")
                lhs_tiles = xv if kind == 2 else xq
                for k in range(4):
                    nc.tensor.matmul(pmm[:, :w_n], lhs_tiles[k][:], wcat[k][:, c0:c1],
                                     start=(k == 0), stop=(k == 3))
                if kind == 0:
                    act.copy(qa_all[:, c0:c1], pmm[:, :w_n])
                elif kind == 1:
                    act.copy(ka_all[:, c0 - NVA:c1 - NVA], pmm[:, :w_n])
                else:
                    v0, v1 = c0 - 2 * NVA, c1 - 2 * NVA
                    if v1 <= NVA:
                        act.copy(va_all[:, v0:v1], pmm[:, :w_n])
                    else:
                        act.copy(va_all[:, v0:NVA], pmm[:, :NVA - v0])
                        vp = pmm[:, NVA - v0:w_n].rearrange("p (h c) -> p h c", h=H)
                        vp_dst = vptok[:, :DH].rearrange("p (h c) -> p h c", h=H)[:, :, :3]
                        act.copy(vp_dst, vp)

            for (c0, c1, kind) in QKV_CHUNKS[:4]:
                qkv_chunk(c0, c1, kind)

            qa_v = qa_all[:].rearrange("p (h q d) -> p h q d", h=H, q=QJ)
            ka_v = ka_all[:].rearrange("p (h k d) -> p h k d", h=H, k=KJ)
            va_v = va_all[:].rearrange("p (h d k) -> p h d k", h=H, d=D)
            att_all = vptok[:, :DH].rearrange("p (h c) -> p h c", h=H)[:, :, 3:]

            # epilogue(i-1) part 1: PE transposes of previous tile's v'
            if pending is not None:
                pvpt, pit = pending
                pvT = []
                for k in range(10):
                    cols = min(P, DH + 1 - k * P)
                    pst2 = ps_t.tile([P, P], hp, tag="pst2")
                    nc.tensor.transpose(pst2[:cols, :], pvpt[:, k * P:k * P + cols], ident[:])
                    pvT.append((pst2, cols))

            # ---- per-tile tiles ----
            GD1 = max(3, HP)                      # D-side p1 group size (also P p1)
            pbigD = bigD.tile([P, GD1 * NP * D], hp, tag="pbigD")
            pbigP = bigP.tile([P, HP * NP * D], hp, tag="pbigP")
            if XPROD:
                pbigX = xb.tile([P, NP * D], hp, tag="pbigX")
            else:
                pbigX = None
            s2aD = bigD.tile([P, HD * NP * 2], hp, tag="s2aD")
            s2aP = bigD.tile([P, HP * NP * 2], hp, tag="s2aP")
            eaD = bigD.tile([P, HD * NP], hp, tag="eaD")
            ebD = bigD.tile([P, HD * NP], hp, tag="ebD")
            eaP = smx.tile([P, HP * NP], hp, tag="eaP")
            ebP = smx.tile([P, HP * NP], hp, tag="ebP")
            eD = (smx if XPROD else bigD).tile([P, HD * NP], hp, tag="eD")
            eP = bigP.tile([P, HP * NP], hp, tag="eP")
            t12D = bigD.tile([P, GD1 * NQK * 12], hp, tag="t12D")
            t6D = small.tile([P, GD1 * NQK * 6], hp, tag="t6D")
            t2D = small.tile([P, GD1 * NQK * 2], hp, tag="t2D")
            t12P = bigP.tile([P, HP * NQK * 12], hp, tag="t12P")
            t6P = bigP.tile([P, HP * NQK * 6], hp, tag="t6P")
            t2P = bigP.tile([P, HP * NQK * 2], hp, tag="t2P")
            a0P = bigP.tile([P, HP * NQK], hp, tag="a0P")
            ctP = bigP.tile([P, HP * D * D], hp, tag="ctP")
            r12D = small.tile([P, HD * QJ * 12], hp, tag="r12D")
            r6D = small.tile([P, HD * QJ * 6], hp, tag="r6D")
            r2D = small.tile([P, HD * QJ * 2], hp, tag="r2D")
            sD = small.tile([P, HD * QJ], f32, tag="sD")
            r16D = small.tile([P, HD * QJ], hp, tag="r16D")
            r12P = bigP.tile([P, HP * QJ * 12], hp, tag="r12P")
            r6P = bigP.tile([P, HP * QJ * 6], hp, tag="r6P")
            r2P = bigP.tile([P, HP * QJ * 2], hp, tag="r2P")
            sP = smx.tile([P, HP * QJ], f32, tag="sP")
            rP16 = smx.tile([P, HP * QJ], hp, tag="rP16")
            a0D = small.tile([P, HD * NQK], hp, tag="a0D")
            ctD = small.tile([P, HD * D * D], hp, tag="ctD")
            uD = small.tile([P, HD * QJ], hp, tag="uD")
            u2D = small.tile([P, HD * QJ], hp, tag="u2D")
            duD = small.tile([P, HD * QJ], hp, tag="duD")
            uP = bigP.tile([P, HP * QJ], hp, tag="uP")
            u2P = bigP.tile([P, HP * QJ], hp, tag="u2P")
            duP = bigP.tile([P, HP * QJ], hp, tag="duP")
            tmp8D = small.tile([P, H * 3], hp, tag="tmp8D")
            tmp8P = bigP.tile([P, H * 3], hp, tag="tmp8P")

            s2Dv = s2aD[:].rearrange("p (h pr e) -> p h pr e", h=HD, pr=NP)
            s2Pv = s2aP[:].rearrange("p (h pr e) -> p h pr e", h=HP, pr=NP)
            eDv = eD[:].rearrange("p (h q k) -> p h q k", h=HD, q=QJ)
            ePv = eP[:].rearrange("p (h q k) -> p h q k", h=HP, q=QJ)

            def p1_s2a(pb, s2t, g0, g1, o, base, full=None):
                nh = g1 - g0
                p1v = pb[:, o * NP * D:(o + nh) * NP * D].rearrange(
                    "p (h q k d) -> p h q k d", h=nh, q=QJ, k=KJ)
                p1f = pb[:, o * NP * D:(o + nh) * NP * D].rearrange(
                    "p (h pr d) -> p h pr d", h=nh, pr=NP)
                qa_b = qa_v[:, g0:g1].unsqueeze(3).broadcast_to([P, nh, QJ, KJ, D])
                ka_b = ka_v[:, g0:g1].unsqueeze(2).broadcast_to([P, nh, QJ, KJ, D])
                dve.tensor_tensor(p1v, qa_b, ka_b, AL.mult)
                rel = g0 - base
                s2v = s2t[:, rel * NP * 2:(rel + nh) * NP * 2].rearrange(
                    "p (h pr e) -> p h pr e", h=nh, pr=NP)
                dve.tensor_tensor(s2v, p1f[:, :, :, 0:2], p1f[:, :, :, 2:4], AL.add)
                dve.tensor_tensor(s2v, s2v, p1f[:, :, :, 4:6], AL.add)
                if full is not None:
                    # full score sum (strided operands -> 1x mode, but frees
                    # the P side from emult on Pool and halves its exps)
                    fv = full[:, rel * NP:(rel + nh) * NP].rearrange(
                        "p (h pr) -> p h pr", h=nh)
                    dve.tensor_tensor(fv, s2v[:, :, :, 0], s2v[:, :, :, 1], AL.add)

            def emult(eng, ea_t, eb_t, e_t, nh):
                eng.tensor_tensor(e_t[:], ea_t[:], eb_t[:], AL.mult)

            def rowsum(eng, e_t, r12t, r6t, r2t, st, nh):
                fq = nh * QJ
                ev = e_t[:].rearrange("p (f k) -> p f k", f=fq)
                r12v = r12t[:].rearrange("p (f k) -> p f k", f=fq)
                r6v = r6t[:].rearrange("p (f k) -> p f k", f=fq)
                r2v = r2t[:].rearrange("p (f k) -> p f k", f=fq)
                eng.tensor_tensor(r12v, ev[:, :, 0:12], ev[:, :, 12:24], AL.add)
                eng.tensor_tensor(r6v, r12v[:, :, 0:6], r12v[:, :, 6:12], AL.add)
                eng.tensor_tensor(r2v, r6v[:, :, 0:2], r6v[:, :, 2:4], AL.add)
                eng.tensor_tensor(r2v, r2v, r6v[:, :, 4:6], AL.add)
                with nc.allow_low_precision(reason="fp16 attention"):
                    eng.tensor_tensor(st[:], r2v[:, :, 0], r2v[:, :, 1], AL.add)

            def chains(eng, resolve, nh, toff, tmp):
                t3 = tmp[:, toff * 3:(toff + nh) * 3].rearrange("p (h c) -> p h c", h=nh)
                for dsl, ssl in CH_LEVELS:
                    nd = dsl[1] - dsl[0]
                    dst = resolve(dsl[0], dsl[1])
                    srcv = resolve(ssl[0], ssl[1])
                    if ssl[1] - ssl[0] < nd:
                        srcv = srcv.broadcast_to([P, nh, nd])
                    eng.tensor_tensor(t3[:, :, :nd], dst, srcv, AL.add)
                    eng.tensor_scalar_mul(dst, t3[:, :, :nd], 0.5)

            def e_row0_resolver(ev):
                return lambda c0, c1: ev[:, :, 0, c0:c1]

            def flat_resolver(v):
                return lambda c0, c1: v[:, :, c0:c1]

            def backend(eng, pb, e_v, g0, g1, t12t, t6t, t2t, a0t, peng_prod=None):
                """PV products + 24->1 k-sum tree, full k24 with fused E."""
                nh = g1 - g0
                prod_eng = peng_prod if peng_prod is not None else eng
                p2v = pb[:, :nh * NQK * KJ].rearrange(
                    "p (h d q k) -> p h d q k", h=nh, d=D, q=QJ)
                e_b = e_v.unsqueeze(2).broadcast_to([P, nh, D, QJ, KJ])
                va_b = va_v[:, g0:g1].unsqueeze(3).broadcast_to([P, nh, D, QJ, KJ])
                prod_eng.tensor_tensor(p2v, e_b, va_b, AL.mult)
                p2f = pb[:, :nh * NQK * KJ].rearrange(
                    "p (h f k) -> p h f k", h=nh, f=NQK)
                t12v = t12t[:, :nh * NQK * 12].rearrange("p (h f k) -> p h f k", h=nh, f=NQK)
                t6v = t6t[:, :nh * NQK * 6].rearrange("p (h f k) -> p h f k", h=nh, f=NQK)
                t2v = t2t[:, :nh * NQK * 2].rearrange("p (h f k) -> p h f k", h=nh, f=NQK)
                eng.tensor_tensor(t12v, p2f[:, :, :, 0:12], p2f[:, :, :, 12:24], AL.add)
                eng.tensor_tensor(t6v, t12v[:, :, :, 0:6], t12v[:, :, :, 6:12], AL.add)
                eng.tensor_tensor(t2v, t6v[:, :, :, 0:2], t6v[:, :, :, 2:4], AL.add)
                eng.tensor_tensor(t2v, t2v, t6v[:, :, :, 4:6], AL.add)
                eng.tensor_tensor(a0t[:, :nh * NQK].rearrange("p (h f) -> p h f", h=nh),
                                  t2v[:, :, :, 0], t2v[:, :, :, 1], AL.add)

            def corr_att(eng, attv, a0v, r_b, du_t, va0, ct_t, nh, r_op=AL.mult):
                eng.tensor_tensor(attv, a0v, r_b, r_op)
                du_v = du_t[:].rearrange("p (h q) -> p h q", h=nh)
                ctv = ct_t[:].rearrange("p (h d q) -> p h d q", h=nh, d=D)
                for qsl, n_q in (((12, 18, 1), 6), ((6, 10, 3), 2)):
                    du_b = du_v[:, :, qsl[0]:qsl[1]:qsl[2]].unsqueeze(2).broadcast_to(
                        [P, nh, D, n_q])
                    va0_b = va0.unsqueeze(3).broadcast_to([P, nh, D, n_q])
                    eng.tensor_tensor(ctv[:, :, :, :n_q], du_b, va0_b, AL.mult)
                    eng.tensor_tensor(attv[:, :, :, qsl[0]:qsl[1]:qsl[2]],
                                      attv[:, :, :, qsl[0]:qsl[1]:qsl[2]],
                                      ctv[:, :, :, :n_q], AL.add)

            # --- DVE front-end: P side first (advances the Pool start gate) ---
            # (p1 scratch goes in pbigD so pbigP stays Pool-only)
            p1_s2a(pbigD, s2aP, HD, H, 0, HD)
            act.activation(eaP[:], s2Pv[:, :, :, 0], AF.Exp)
            act.activation(ebP[:], s2Pv[:, :, :, 1], AF.Exp)
            # remaining (D-side-only) q/k chunks, behind the P exps on ACT
            for (c0, c1, kind) in QKV_CHUNKS[4:6]:
                qkv_chunk(c0, c1, kind)

            # Pool picks up the P side from emult onward
            emult(pe_em, eaP, ebP, eP, HP)
            rowsum(pe_rs, eP, r12P, r6P, r2P, sP, HP)
            chains(pe_ce, e_row0_resolver(ePv), HP, 0, tmp8P)
            rPv, rP_op = rP16[:].rearrange("p (h q) -> p h q", h=HP), AL.mult

            # --- DVE D-side front-end (boundary head HD-1 first for XPROD) ---
            p1_s2a(pbigD, s2aD, HD - GD1, HD, 0, 0)
            act.activation(eaD[:, (HD - GD1) * NP:], s2Dv[:, HD - GD1:, :, 0], AF.Exp)
            act.activation(ebD[:, (HD - GD1) * NP:], s2Dv[:, HD - GD1:, :, 1], AF.Exp)
            if HD > GD1:
                p1_s2a(pbigD, s2aD, 0, HD - GD1, 0, 0)
                act.activation(eaD[:, :(HD - GD1) * NP], s2Dv[:, :HD - GD1, :, 0], AF.Exp)
                act.activation(ebD[:, :(HD - GD1) * NP], s2Dv[:, :HD - GD1, :, 1], AF.Exp)

            # va chunks (ACT after the exps) -- Pool p2 needs them
            for (c0, c1, kind) in QKV_CHUNKS[6:]:
                qkv_chunk(c0, c1, kind)

            # D-side emult + rowsum (rowsum must see pre-chain E), then chains;
            # split into the p1 groups so DVE starts on g1 while ACT exps g2
            def emult_g(g0, g1):
                dve.tensor_tensor(eD[:, g0 * NP:g1 * NP],
                                  eaD[:, g0 * NP:g1 * NP],
                                  ebD[:, g0 * NP:g1 * NP], AL.mult)

            def rowsum_g(g0, g1):
                fq0, fq1 = g0 * QJ, g1 * QJ
                ev = eD[:, g0 * NP:g1 * NP].rearrange("p (f k) -> p f k", k=KJ)
                r12v = r12D[:, fq0 * 12:fq1 * 12].rearrange("p (f k) -> p f k", k=12)
                r6v = r6D[:, fq0 * 6:fq1 * 6].rearrange("p (f k) -> p f k", k=6)
                r2v = r2D[:, fq0 * 2:fq1 * 2].rearrange("p (f k) -> p f k", k=2)
                dve.tensor_tensor(r12v, ev[:, :, 0:12], ev[:, :, 12:24], AL.add)
                dve.tensor_tensor(r6v, r12v[:, :, 0:6], r12v[:, :, 6:12], AL.add)
                dve.tensor_tensor(r2v, r6v[:, :, 0:2], r6v[:, :, 2:4], AL.add)
                dve.tensor_tensor(r2v, r2v, r6v[:, :, 4:6], AL.add)
                with nc.allow_low_precision(reason="fp16 attention"):
                    dve.tensor_tensor(sD[:, fq0:fq1], r2v[:, :, 0], r2v[:, :, 1],
                                      AL.add)

            emult_g(HD - GD1, HD)
            rowsum_g(HD - GD1, HD)
            if HD > GD1:
                emult_g(0, HD - GD1)
                rowsum_g(0, HD - GD1)
            chains(dve, e_row0_resolver(eDv), HD, 0, tmp8D)

            # epilogue(i-1) part 2: vT copies (ACT), final matmul, out
            if pending is not None:
                vT = []
                for k in range(10):
                    pst2, cols = pvT[k]
                    vk = vt.tile([P, P], hp, tag=f"vT{k}")
                    act.copy(vk[:cols, :], pst2[:cols, :])
                    vT.append((vk, cols))
                pout = ps_out.tile([P, D_MODEL], f32, tag="pout")
                for k in range(10):
                    vk, rows = vT[k]
                    lwk, rows2 = lw[k]
                    nc.tensor.matmul(pout[:], vk[:rows, :], lwk[:rows, :],
                                     start=(k == 0), stop=(k == 9))
                out_sb = outp.tile([P, D_MODEL], f32, tag="out_sb")
                act.copy(out_sb[:], pout[:])
                nc.sync.dma_start(out_dram[pit * P:(pit + 1) * P, :], out_sb[:])

            # Pool backend (P side), full k24
            backend(pe_bk, pbigP, ePv, HD, H, t12P, t6P, t2P, a0P,
                    peng_prod=pe_bp)
            # u-path + P-side 1/r on DVE's slack window: sP/eP are
            # Pool-produced well before DVE reaches this point, and Pool
            # consumes rP16/duP only at its corr tail (Pool is the binding
            # engine here).
            with nc.allow_low_precision(reason="fp16 attention"):
                dve.reciprocal(rP16[:], sP[:])
            dve.tensor_tensor(uP[:].rearrange("p (h q) -> p h q", h=HP),
                              ePv[:, :, :, 0], rPv, AL.mult)
            dve.tensor_copy(u2P[:], uP[:])
            chains(dve, flat_resolver(u2P[:].rearrange("p (h q) -> p h q", h=HP)),
                   HP, HP, tmp8P)
            dve.tensor_tensor(duP[:], u2P[:], uP[:], AL.subtract)
            a0Pv = a0P[:].rearrange("p (h d q) -> p h d q", h=HP, d=D)
            attP = att_all[:, HD:].rearrange("p h (d q) -> p h d q", d=D)
            r_bP = rPv.unsqueeze(2).broadcast_to([P, HP, D, QJ])
            corr_att(pe_bk, attP, a0Pv, r_bP, duP, va_v[:, HD:, :, 0], ctP, HP,
                     r_op=rP_op)

            # --- DVE D-side rest ---
            with nc.allow_low_precision(reason="fp16 attention"):
                dve.reciprocal(r16D[:], sD[:])
            rDv = r16D[:].rearrange("p (h q) -> p h q", h=HD)
            dve.tensor_tensor(uD[:].rearrange("p (h q) -> p h q", h=HD),
                              eDv[:, :, :, 0], rDv, AL.mult)
            dve.tensor_copy(u2D[:], uD[:])
            chains(dve, flat_resolver(u2D[:].rearrange("p (h q) -> p h q", h=HD)),
                   HD, 0, tmp8D)
            dve.tensor_tensor(duD[:], u2D[:], uD[:], AL.subtract)

            # D-side backends: boundary head's products optionally on Pool
            xh = HD - 1
            if XPROD:
                backend(dve, pbigX, eDv[:, xh:HD], xh, HD, t12D, t6D, t2D,
                        a0D[:, xh * NQK:], peng_prod=pe_xp)
            gsz = HD - (1 if XPROD else 0)
            done = 0
            while done < gsz:
                g = min(GD1, gsz - done)
                off = done * NQK
                a0s = a0D[:, off:off + g * NQK]
                backend(dve, pbigD, eDv[:, done:done + g], done, done + g,
                        t12D, t6D, t2D, a0s)
                done += g
            a0Dv = a0D[:].rearrange("p (h d q) -> p h d q", h=HD, d=D)
            attD = att_all[:, :HD].rearrange("p h (d q) -> p h d q", d=D)
            r_bD = rDv.unsqueeze(2).broadcast_to([P, HD, D, QJ])
            corr_att(dve, attD, a0Dv, r_bD, duD, va_v[:, :HD, :, 0], ctD, HD)

            pending = (vptok, it)

        # epilogue flush for the last tile
        pvpt, pit = pending
        vT = []
        for k in range(10):
            cols = min(P, DH + 1 - k * P)
            pst2 = ps_t.tile([P, P], hp, tag="pst2")
            nc.tensor.transpose(pst2[:cols, :], pvpt[:, k * P:k * P + cols], ident[:])
            vk = vt.tile([P, P], hp, tag=f"vT{k}")
            act.copy(vk[:cols, :], pst2[:cols, :])
            vT.append((vk, cols))
        pout = ps_out.tile([P, D_MODEL], f32, tag="pout")
        for k in range(10):
            vk, rows = vT[k]
            lwk, rows2 = lw[k]
            nc.tensor.matmul(pout[:], vk[:rows, :], lwk[:rows, :],
                             start=(k == 0), stop=(k == 9))
        out_sb = outp.tile([P, D_MODEL], f32, tag="out_sb")
        act.copy(out_sb[:], pout[:])
        nc.sync.dma_start(out_dram[pit * P:(pit + 1) * P, :], out_sb[:])

    nc.compile()
    return nc


def prep_weights(qk_w, v_w, lin_w, lin_b):
    scale = np.float32(1.0 / np.sqrt(6.0))
    wq = np.asarray(qk_w[:, :DH], dtype=np.float32).reshape(D_MODEL, H, 147)
    wk = np.asarray(qk_w[:, DH:], dtype=np.float32).reshape(D_MODEL, H, 147)
    wv = np.asarray(v_w, dtype=np.float32).reshape(D_MODEL, H, 147)
    wq_p = (wq[:, :, 3:] * scale).reshape(D_MODEL, H * NQK)
    wk_p = wk[:, :, 3:].reshape(D_MODEL, H * NQK)
    wv_att = wv[:, :, 3:].reshape(D_MODEL, H, KJ, D).transpose(0, 1, 3, 2).reshape(D_MODEL, H * NQK)
    wv_pass = wv[:, :, :3].reshape(D_MODEL, H * 3)
    w_cat = np.ascontiguousarray(
        np.concatenate([wq_p, wk_p, wv_att, wv_pass], axis=1)).astype(np.float16)
    # lin_w rows permuted to the (h, [pass3, d*24+q]) vptok layout + bias row
    lwr = np.asarray(lin_w, dtype=np.float32).reshape(H, 147, D_MODEL)
    att = lwr[:, 3:, :].reshape(H, QJ, D, D_MODEL).transpose(0, 2, 1, 3).reshape(H, NQK, D_MODEL)
    lw_p = np.concatenate([lwr[:, :3, :], att], axis=1).reshape(DH, D_MODEL)
    lw_aug = np.zeros((1184, D_MODEL), dtype=np.float32)
    lw_aug[:DH] = lw_p
    lw_aug[DH] = np.asarray(lin_b, dtype=np.float32)
    return w_cat, np.ascontiguousarray(lw_aug).astype(np.float16)


def make_in_maps(query, value, qk_w, v_w, lin_w, lin_b):
    w_cat, lw_aug = prep_weights(qk_w, v_w, lin_w, lin_b)
    q = np.asarray(query, dtype=np.float32)
    v = np.asarray(value, dtype=np.float32)
    bpc = B // N_CORES
    in_maps = []
    for c in range(N_CORES):
        qc = q[c * bpc:(c + 1) * bpc].reshape(-1, D_MODEL).T
        vc = v[c * bpc:(c + 1) * bpc].reshape(-1, D_MODEL).T
        in_maps.append({
            "qT": np.ascontiguousarray(qc).astype(np.float16),
            "vT": np.ascontiguousarray(vc).astype(np.float16),
            "w_cat": w_cat,
            "lin_w": lw_aug,
        })
    return in_maps


_CACHED_NC = None


def _get_nc():
    global _CACHED_NC
    if _CACHED_NC is None:
        _CACHED_NC = build_program(TT)
    return _CACHED_NC


def kernel(query, key, value, qk_w, v_w, lin_w, lin_b, _want_results=False, **_ignored):
    """Full-input kernel: shards batch over 8 cores, returns full output."""
    in_maps = make_in_maps(query, value, qk_w, v_w, lin_w, lin_b)
    nc = _get_nc()
    bpc = B // N_CORES
    res = run_bass_kernel_spmd(nc, in_maps, core_ids=list(range(N_CORES)))
    out = np.empty((B, N, D_MODEL), dtype=np.float32)
    for c in range(N_CORES):
        out[c * bpc:(c + 1) * bpc] = res.results[c]["out"].reshape(bpc, N, D_MODEL)
    if _want_results:
        return out, res
    return out
